# revision 10
# baseline (speedup 1.0000x reference)
"""Bernstein flow density kernel for 8x TRN2 NeuronCores.

Math (per sample n):
  density(n) = prod_i [ phi_i[n,15] + sum_m tf_i[n,m] * psi_i[n,m] ]
  tf_i = cond_i @ c_alpha_i,  cond_i = B_0 (x) ... (x) B_{i-1}  (row-wise Kron)
Key identity: Bernstein bases sum to 1, so cond_i is a marginal of
cond_5 [N,1024]; all six matmuls merge into ONE:
  tf_all[N, 90] = cond_5 @ W,  W[c, i*15+m] = c_alpha_i[c >> 2*(5-i), m]
psi_i[n,m] = phi_i[n,m] - phi_i[n,m+1] (m=0..14), phi = scaled Bernstein deg-15.

Per core (8192 samples, p-major: local n = p*64 + s):
  1. build deg-3 factor tables B_j [128,(s,j,a)] with vector ops
  2. per s-tile: cond_5 [128,1024] via 4 broadcast-AP tensor_tensor ops
  3. PE-transpose 128x128 blocks -> cond^T chunks; fp32 matmul vs W -> tf^T
  4. PE-transpose tf^T back to natural; build phi/psi; combine + 6-way product
"""

import math
import sys

import numpy as np

sys.path.insert(0, "/opt/trn_rl_repo")

import concourse.bacc as bacc  # noqa: E402
import concourse.bass as bass  # noqa: E402
import concourse.tile as tile  # noqa: E402
from concourse import mybir  # noqa: E402
from concourse.bass_utils import run_bass_kernel_spmd  # noqa: E402

N = 65536
DIM = 6
NCORES = 8
NC = N // NCORES          # 8192 samples per core
P = 128
S = NC // P               # 64 samples per partition
NT = 4                    # s-tiles per matmul group
NG = S // NT              # 16 groups (matmul chunks of 512 samples)
NB = NT * P               # 512 samples per group
CDIM = 1024               # cond_5 width
KCH = CDIM // P           # 8 contraction chunks
M90 = 90                  # 6 dims * 15 coeffs

F32 = mybir.dt.float32
MUL = mybir.AluOpType.mult
ADD = mybir.AluOpType.add
SUB = mybir.AluOpType.subtract

_CACHE = {}


def _ap(a, off_elems, dims):
    """AP over slice a with replaced free dims; dims = [[step,count],...]."""
    return bass.AP(tensor=a.tensor, offset=a.offset + off_elems, ap=[a.ap[0]] + dims)


def _build_nc(mm_dtype=F32):
    nc = bacc.Bacc(target_bir_lowering=False, trn_type="TRN2")

    xr = nc.dram_tensor("xr", [P, S, DIM], F32, kind="ExternalInput")
    wmat = nc.dram_tensor("wmat", [CDIM, M90], F32, kind="ExternalInput")
    kap = nc.dram_tensor("kap", [1, 16], F32, kind="ExternalInput")
    ident = nc.dram_tensor("ident", [P, P], F32, kind="ExternalInput")
    dens_out = nc.dram_tensor("dens", [P, S], F32, kind="ExternalOutput")

    with tile.TileContext(nc) as tc:
        with (
            tc.tile_pool(name="singles", bufs=1) as singles,
            tc.tile_pool(name="bigs", bufs=1) as bigs,
            tc.tile_pool(name="cond", bufs=2) as condp,
            tc.tile_pool(name="ctb", bufs=2) as ctbp,
            tc.tile_pool(name="pows", bufs=2) as powp,
            tc.tile_pool(name="ps_t", bufs=2, space="PSUM") as ps_t,
            tc.tile_pool(name="ps_mm", bufs=2, space="PSUM") as ps_mm,
            tc.tile_pool(name="ps_d", bufs=1, space="PSUM") as ps_d,
            tc.tile_pool(name="ps_x", bufs=1, space="PSUM") as ps_x,
        ):
            # ---- constants / inputs ----
            xin = singles.tile([P, S, DIM], F32)
            nc.sync.dma_start(out=xin[:, :, :], in_=xr[:, :, :])
            wsb = singles.tile([P, KCH, M90], F32)
            nc.sync.dma_start(
                out=wsb[:, :, :],
                in_=bass.AP(tensor=wmat[:, :].tensor, offset=0,
                            ap=[[M90, P], [P * M90, KCH], [1, M90]]),
            )
            idn = singles.tile([P, P], F32)
            nc.sync.dma_start(out=idn[:, :], in_=ident[:, :])
            kapt = singles.tile([P, 16], F32)
            nc.sync.dma_start(
                out=kapt[:, :],
                in_=bass.AP(tensor=kap[:, :].tensor, offset=0, ap=[[0, P], [1, 16]]),
            )

            # PE "pre-observe" dummies: walrus fp32 fused matmul (LDW+MM)
            # tolerates only one sync wait, so make the PE observe the DMA
            # semaphores up front via tiny throwaway transposes.
            scr = ps_x.tile([2, 2], F32)
            nc.tensor.matmul(out=scr[:, :], lhsT=idn[:2, :2], rhs=idn[:2, :2],
                             is_transpose=True, start=True, stop=True,
                             skip_group_check=True)
            nc.tensor.matmul(out=scr[:, :], lhsT=wsb[:2, 0, :2], rhs=idn[:2, :2],
                             is_transpose=True, start=True, stop=True,
                             skip_group_check=True)

            xa = xin[:, :, :]

            # ---- stage A: powers of x, 1-x ----
            FD6 = S * DIM
            omx = singles.tile([P, S, DIM], F32)
            x2 = singles.tile([P, S, DIM], F32)
            x3 = singles.tile([P, S, DIM], F32)
            omx2 = singles.tile([P, S, DIM], F32)
            omx3 = singles.tile([P, S, DIM], F32)
            # omx = (x * -1) + 1
            nc.vector.tensor_scalar(
                out=omx[:, :, :], in0=xa, scalar1=-1.0, scalar2=1.0, op0=MUL, op1=ADD
            )
            nc.vector.tensor_tensor(out=x2[:, :, :], in0=xa, in1=xa, op=MUL)
            nc.vector.tensor_tensor(
                out=omx2[:, :, :], in0=omx[:, :, :], in1=omx[:, :, :], op=MUL
            )
            nc.vector.tensor_tensor(out=x3[:, :, :], in0=x2[:, :, :], in1=xa, op=MUL)
            nc.vector.tensor_tensor(
                out=omx3[:, :, :], in0=omx2[:, :, :], in1=omx[:, :, :], op=MUL
            )

            # ---- stage B: deg-3 tables Bbig[p, s, j, a]  j=0..4 ----
            NJ = 5
            Bbig = singles.tile([P, S, NJ, 4], F32)
            for (a, src, scl, other) in (
                (0, omx3, None, None),
                (1, xin, 3.0, omx2),
                (2, x2, 3.0, omx),
                (3, x3, None, None),
            ):
                src_ap = _ap(src[:, :, :], 0, [[DIM, S], [1, NJ]])
                out_ap = _ap(Bbig[:, :, :, :], a, [[4 * NJ, S], [4, NJ]])
                if scl is None:
                    nc.vector.tensor_copy(out=out_ap, in_=src_ap)
                else:
                    nc.vector.scalar_tensor_tensor(
                        out=out_ap, in0=src_ap, scalar=scl,
                        in1=_ap(other[:, :, :], 0, [[DIM, S], [1, NJ]]),
                        op0=MUL, op1=MUL,
                    )

            # ---- stage C+D: cond tiles, transpose, matmul per group ----
            tf_big = bigs.tile([P, S, M90], F32)   # natural-layout tf
            for g in range(NG):
                ctb = ctbp.tile([P, KCH, NT, P], F32, tag="ctb")
                for t in range(NT):
                    s = g * NT + t
                    cnd = condp.tile([P, CDIM], F32, tag="cond")
                    k2 = condp.tile([P, 16], F32, tag="k2")
                    k3 = condp.tile([P, 64], F32, tag="k3")
                    k4 = condp.tile([P, 256], F32, tag="k4")
                    boff = s * NJ * 4
                    bb = Bbig[:, :, :, :]

                    def bj(j, rep, tilec):
                        # B_j values: [[0,rep],[1,4]] tiled -> broadcast block
                        return _ap(bb, boff + j * 4, [[0, rep], [1, 4]]) if tilec \
                            else _ap(bb, boff + j * 4, [[1, 4], [0, rep]])

                    nc.vector.tensor_tensor(
                        out=k2[:, :], in0=bj(0, 4, False), in1=bj(1, 4, True), op=MUL)
                    nc.vector.tensor_tensor(
                        out=k3[:, :],
                        in0=_ap(k2[:, :], 0, [[1, 16], [0, 4]]),
                        in1=bj(2, 16, True), op=MUL)
                    nc.vector.tensor_tensor(
                        out=k4[:, :],
                        in0=_ap(k3[:, :], 0, [[1, 64], [0, 4]]),
                        in1=bj(3, 64, True), op=MUL)
                    nc.vector.tensor_tensor(
                        out=cnd[:, :],
                        in0=_ap(k4[:, :], 0, [[1, 256], [0, 4]]),
                        in1=bj(4, 256, True), op=MUL)

                    # transpose 8 128x128 blocks -> 2-bank psum tile
                    pst = ps_t.tile([P, KCH, P], F32, tag="pst")
                    # dummy absorbs the psum-slot-release wait so the first
                    # real transpose carries only the DVE (cond) wait
                    nc.tensor.matmul(out=pst[:2, 0, :2], lhsT=idn[:2, :2],
                                     rhs=idn[:2, :2], is_transpose=True,
                                     start=True, stop=True,
                                     skip_group_check=True)
                    for k in range(KCH):
                        nc.tensor.matmul(
                            out=pst[:, k, :],
                            lhsT=cnd[:, k * P:(k + 1) * P],
                            rhs=idn[:, :],
                            is_transpose=True,
                            start=(k % 4 == 0),
                            stop=(k % 4 == 3),
                        )
                    # copy psum -> condT sbuf (scatter over k, col t*128)
                    for half in range(2):
                        nc.scalar.copy(
                            out=_ap(ctb[:, :, :, :], (half * 4) * NT * P + t * P,
                                    [[NT * P, 4], [1, P]]),
                            in_=_ap(pst[:, :, :], half * 4 * P, [[P, 4], [1, P]]),
                        )

                pmm = ps_mm.tile([M90, NB], F32, tag="pmm")
                for k in range(KCH):
                    nc.tensor.matmul(
                        out=pmm[:, :],
                        lhsT=wsb[:, k, :],
                        rhs=ctb[:, k, :, :],
                        start=(k == 0),
                        stop=(k == KCH - 1),
                    )
                tft = ctbp.tile([M90, NB], F32, tag="tft")
                nc.scalar.copy(out=tft[:, :], in_=pmm[:, :])
                # de-transpose tf^T -> natural [128, t, 90]
                psd = ps_d.tile([P, NT, M90], F32, tag="psd")
                for t in range(NT):
                    nc.tensor.matmul(
                        out=psd[:, t, :],
                        lhsT=tft[:, t * P:(t + 1) * P],
                        rhs=idn[:M90, :M90],
                        is_transpose=True,
                        start=(t == 0),
                        stop=(t == NT - 1),
                    )
                # scatter to tf_big[p, s, :], s = g*NT + t
                nc.scalar.copy(
                    out=_ap(tf_big[:, :, :], g * NT * M90, [[1, NT * M90]]),
                    in_=_ap(psd[:, :, :], 0, [[1, NT * M90]]),
                )

            # ---- stage E: phi/psi ----
            psi_big = bigs.tile([P, S, DIM, 15], F32)
            p15 = bigs.tile([P, S, DIM], F32)
            for j in range(DIM):
                px = powp.tile([P, 16, S], F32, tag="px")
                pq = powp.tile([P, 16, S], F32, tag="pq")
                for (tbl, base) in ((px, xin), (pq, omx)):
                    nc.vector.memset(tbl[:, 0, :], 1.0)
                    nc.vector.tensor_copy(
                        out=tbl[:, 1, :], in_=_ap(base[:, :, :], j, [[DIM, S]]))
                    t1 = tbl[:, :, :]
                    nc.vector.tensor_tensor(
                        out=tbl[:, 2, :], in0=t1[:, 1, :], in1=t1[:, 1, :], op=MUL)
                    nc.vector.tensor_tensor(
                        out=_ap(t1, 3 * S, [[1, 2 * S]]),
                        in0=_ap(t1, S, [[1, 2 * S]]),
                        in1=_ap(t1, 2 * S, [[0, 2], [1, S]]), op=MUL)
                    nc.vector.tensor_tensor(
                        out=_ap(t1, 5 * S, [[1, 4 * S]]),
                        in0=_ap(t1, S, [[1, 4 * S]]),
                        in1=_ap(t1, 4 * S, [[0, 4], [1, S]]), op=MUL)
                    nc.vector.tensor_tensor(
                        out=_ap(t1, 9 * S, [[1, 7 * S]]),
                        in0=_ap(t1, S, [[1, 7 * S]]),
                        in1=_ap(t1, 8 * S, [[0, 7], [1, S]]), op=MUL)
                # v = kappa[a] * x^a * (1-x)^(15-a)   (a-major [16, S])
                v = powp.tile([P, 16, S], F32, tag="v")
                nc.vector.tensor_tensor(
                    out=v[:, :, :],
                    in0=px[:, :, :],
                    in1=_ap(pq[:, :, :], 15 * S, [[-S, 16], [1, S]]), op=MUL)
                nc.vector.tensor_tensor(
                    out=v[:, :, :], in0=v[:, :, :],
                    in1=_ap(kapt[:, :], 0, [[1, 16], [0, S]]), op=MUL)
                # psi[p, s, j, m] = v[m, s] - v[m+1, s]
                nc.vector.tensor_tensor(
                    out=_ap(psi_big[:, :, :, :], j * 15, [[DIM * 15, S], [1, 15]]),
                    in0=_ap(v[:, :, :], 0, [[1, S], [S, 15]]),
                    in1=_ap(v[:, :, :], S, [[1, S], [S, 15]]), op=SUB)
                nc.vector.tensor_copy(
                    out=_ap(p15[:, :, :], j, [[DIM, S]]),
                    in_=_ap(v[:, :, :], 15 * S, [[1, S]]))

            # ---- stage F: combine ----
            ebig = bigs.tile([P, S, DIM, 15], F32)
            nc.vector.tensor_tensor(
                out=ebig[:, :, :, :],
                in0=_ap(tf_big[:, :, :], 0, [[1, S * DIM * 15]]),
                in1=_ap(psi_big[:, :, :, :], 0, [[1, S * DIM * 15]]), op=MUL)
            sred = bigs.tile([P, S, DIM], F32)
            nc.vector.tensor_reduce(
                out=sred[:, :, :],
                in_=_ap(ebig[:, :, :, :], 0, [[15, S * DIM], [1, 15]]),
                op=ADD, axis=mybir.AxisListType.X)
            nc.vector.tensor_tensor(
                out=sred[:, :, :], in0=sred[:, :, :], in1=p15[:, :, :], op=ADD)
            t1 = bigs.tile([P, S, 3], F32)
            nc.vector.tensor_tensor(
                out=t1[:, :, :],
                in0=_ap(sred[:, :, :], 0, [[DIM, S], [2, 3]]),
                in1=_ap(sred[:, :, :], 1, [[DIM, S], [2, 3]]), op=MUL)
            dq = bigs.tile([P, S], F32)
            nc.vector.tensor_tensor(
                out=dq[:, :],
                in0=_ap(t1[:, :, :], 0, [[3, S]]),
                in1=_ap(t1[:, :, :], 1, [[3, S]]), op=MUL)
            nc.vector.tensor_tensor(
                out=dq[:, :], in0=dq[:, :],
                in1=_ap(t1[:, :, :], 2, [[3, S]]), op=MUL)
            nc.sync.dma_start(out=dens_out[:, :], in_=dq[:, :])

    nc.finalize()
    return nc


def _softplus64(v):
    return np.logaddexp(0.0, v)


def _host_w(As):
    cols = []
    for i in range(DIM):
        c = np.cumsum(_softplus64(As[i].astype(np.float64)), axis=1)
        ca = 2.0 * (1.0 / (1.0 + np.exp(-c)) - 0.5)
        cols.append(np.repeat(ca, 4 ** (5 - i), axis=0))
    return np.concatenate(cols, axis=1).astype(np.float32)


def kernel(**inputs):
    x = np.asarray(inputs["x"], dtype=np.float32)
    As = [np.asarray(inputs[f"A{i}"], dtype=np.float32) for i in range(DIM)]

    if "nc" not in _CACHE:
        _CACHE["nc"] = _build_nc()
    nc = _CACHE["nc"]

    w = _host_w(As)
    kapv = (16.0 * np.array([math.comb(15, a) for a in range(16)],
                            dtype=np.float64)).astype(np.float32)[None, :]
    idn = np.eye(P, dtype=np.float32)

    in_maps = []
    for c in range(NCORES):
        xc = x[c * NC:(c + 1) * NC].reshape(P, S, DIM)
        in_maps.append({"xr": xc, "wmat": w, "kap": kapv, "ident": idn})

    res = run_bass_kernel_spmd(nc, in_maps, core_ids=list(range(NCORES)))
    outs = [r["dens"].reshape(NC) for r in res.results]
    return np.concatenate(outs, axis=0)


if __name__ == "__main__":
    rng = np.random.default_rng(0)
    ins = {"x": rng.uniform(0, 1, (N, DIM)).astype(np.float32)}
    for i in range(DIM):
        ins[f"A{i}"] = rng.uniform(0, 1, ((4 ** i), 15)).astype(np.float32)
    out = kernel(**ins)
    print(out.shape, out[:4])


# revision 18
# speedup vs baseline: 1.0062x; 1.0062x over previous
"""Bernstein flow density kernel for 8x TRN2 NeuronCores.

Math (per sample n):
  density(n) = prod_i [ phi_i[n,15] + sum_m tf_i[n,m] * psi_i[n,m] ]
  tf_i = cond_i @ c_alpha_i,  cond_i = B_0 (x) ... (x) B_{i-1}  (row-wise Kron)
Key identity: Bernstein bases sum to 1, so cond_i is a marginal of
cond_5 [N,1024]; all six matmuls merge into ONE:
  tf_all[N, 90] = cond_5 @ W,  W[c, i*15+m] = c_alpha_i[c >> 2*(5-i), m]
psi_i[n,m] = phi_i[n,m] - phi_i[n,m+1] (m=0..14), phi = scaled Bernstein deg-15.

Per core (8192 samples, p-major: local n = p*64 + s):
  1. build deg-3 factor tables B_j [128,(s,j,a)] with vector ops
  2. per s-tile: cond_5 [128,1024] via 4 broadcast-AP tensor_tensor ops
  3. PE-transpose 128x128 blocks -> cond^T chunks; fp32 matmul vs W -> tf^T
  4. PE-transpose tf^T back to natural; build phi/psi; combine + 6-way product
"""

import math
import sys

import numpy as np

sys.path.insert(0, "/opt/trn_rl_repo")

import concourse.bacc as bacc  # noqa: E402
import concourse.bass as bass  # noqa: E402
import concourse.tile as tile  # noqa: E402
from concourse import mybir  # noqa: E402
from concourse.bass_utils import run_bass_kernel_spmd  # noqa: E402

N = 65536
DIM = 6
NCORES = 8
NC = N // NCORES          # 8192 samples per core
P = 128
S = NC // P               # 64 samples per partition
NT = 4                    # s-tiles per matmul group
NG = S // NT              # 16 groups (matmul chunks of 512 samples)
NB = NT * P               # 512 samples per group
CDIM = 1024               # cond_5 width
KCH = CDIM // P           # 8 contraction chunks
M90 = 90                  # 6 dims * 15 coeffs

F32 = mybir.dt.float32
F32R = mybir.dt.float32r
MUL = mybir.AluOpType.mult
ADD = mybir.AluOpType.add
SUB = mybir.AluOpType.subtract

_CACHE = {}


def _ap(a, off_elems, dims):
    """AP over slice a with replaced free dims; dims = [[step,count],...]."""
    return bass.AP(tensor=a.tensor, offset=a.offset + off_elems, ap=[a.ap[0]] + dims)


def _build_nc(mm_dtype=F32):
    nc = bacc.Bacc(target_bir_lowering=False, trn_type="TRN2")

    xr = nc.dram_tensor("xr", [P, S, DIM], F32, kind="ExternalInput")
    wmat = nc.dram_tensor("wmat", [CDIM, M90], F32, kind="ExternalInput")
    kap = nc.dram_tensor("kap", [1, 16], F32, kind="ExternalInput")
    ident = nc.dram_tensor("ident", [P, P], F32, kind="ExternalInput")
    dens_out = nc.dram_tensor("dens", [P, S], F32, kind="ExternalOutput")

    with tile.TileContext(nc) as tc:
        with (
            tc.tile_pool(name="singles", bufs=1) as singles,
            tc.tile_pool(name="bigs", bufs=1) as bigs,
            tc.tile_pool(name="cond", bufs=2) as condp,
            tc.tile_pool(name="ctb", bufs=2) as ctbp,
            tc.tile_pool(name="pows", bufs=2) as powp,
            tc.tile_pool(name="ps_t", bufs=2, space="PSUM") as ps_t,
            tc.tile_pool(name="ps_mm", bufs=2, space="PSUM") as ps_mm,
            tc.tile_pool(name="ps_d", bufs=1, space="PSUM") as ps_d,
            tc.tile_pool(name="ps_x", bufs=1, space="PSUM") as ps_x,
        ):
            # ---- constants / inputs ----
            xin = singles.tile([P, S, DIM], F32)
            nc.sync.dma_start(out=xin[:, :, :], in_=xr[:, :, :])
            wsb = singles.tile([P, KCH, M90], F32)
            nc.sync.dma_start(
                out=wsb[:, :, :],
                in_=bass.AP(tensor=wmat[:, :].tensor, offset=0,
                            ap=[[M90, P], [P * M90, KCH], [1, M90]]),
            )
            idn = singles.tile([P, P], F32)
            nc.sync.dma_start(out=idn[:, :], in_=ident[:, :])
            kapt = singles.tile([P, 16], F32)
            nc.sync.dma_start(
                out=kapt[:, :],
                in_=bass.AP(tensor=kap[:, :].tensor, offset=0, ap=[[0, P], [1, 16]]),
            )

            # PE "pre-observe" dummies: walrus fp32 fused matmul (LDW+MM)
            # tolerates only one sync wait, so make the PE observe the DMA
            # semaphores up front via tiny throwaway transposes.
            scr = ps_x.tile([2, 2], F32)
            nc.tensor.matmul(out=scr[:, :], lhsT=idn[:2, :2], rhs=idn[:2, :2],
                             is_transpose=True, start=True, stop=True,
                             skip_group_check=True)
            nc.tensor.matmul(out=scr[:, :], lhsT=wsb[:2, 0, :2], rhs=idn[:2, :2],
                             is_transpose=True, start=True, stop=True,
                             skip_group_check=True)

            xa = xin[:, :, :]

            # ---- stage A: powers of x, 1-x ----
            FD6 = S * DIM
            omx = singles.tile([P, S, DIM], F32)
            x2 = singles.tile([P, S, DIM], F32)
            x3 = singles.tile([P, S, DIM], F32)
            omx2 = singles.tile([P, S, DIM], F32)
            omx3 = singles.tile([P, S, DIM], F32)
            # omx = (x * -1) + 1
            nc.vector.tensor_scalar(
                out=omx[:, :, :], in0=xa, scalar1=-1.0, scalar2=1.0, op0=MUL, op1=ADD
            )
            nc.vector.tensor_tensor(out=x2[:, :, :], in0=xa, in1=xa, op=MUL)
            nc.vector.tensor_tensor(
                out=omx2[:, :, :], in0=omx[:, :, :], in1=omx[:, :, :], op=MUL
            )
            nc.vector.tensor_tensor(out=x3[:, :, :], in0=x2[:, :, :], in1=xa, op=MUL)
            nc.vector.tensor_tensor(
                out=omx3[:, :, :], in0=omx2[:, :, :], in1=omx[:, :, :], op=MUL
            )

            # ---- stage B: deg-3 tables Bbig[p, s, j, a]  j=0..4 ----
            NJ = 5
            Bbig = singles.tile([P, S, NJ, 4], F32)
            for (a, src, scl, other) in (
                (0, omx3, None, None),
                (1, xin, 3.0, omx2),
                (2, x2, 3.0, omx),
                (3, x3, None, None),
            ):
                src_ap = _ap(src[:, :, :], 0, [[DIM, S], [1, NJ]])
                out_ap = _ap(Bbig[:, :, :, :], a, [[4 * NJ, S], [4, NJ]])
                if scl is None:
                    nc.vector.tensor_copy(out=out_ap, in_=src_ap)
                else:
                    nc.vector.scalar_tensor_tensor(
                        out=out_ap, in0=src_ap, scalar=scl,
                        in1=_ap(other[:, :, :], 0, [[DIM, S], [1, NJ]]),
                        op0=MUL, op1=MUL,
                    )

            # ---- stage C+D: cond tiles, transpose, matmul per group ----
            tf_big = bigs.tile([P, S, M90], F32)   # natural-layout tf
            for g in range(NG):
                ctb = ctbp.tile([P, KCH, NT, P], F32, tag="ctb")
                for t in range(NT):
                    s = g * NT + t
                    cnd = condp.tile([P, CDIM], F32, tag="cond")
                    k2 = condp.tile([P, 16], F32, tag="k2")
                    k3 = condp.tile([P, 64], F32, tag="k3")
                    q34 = condp.tile([P, 16], F32, tag="q34")
                    boff = s * NJ * 4
                    bb = Bbig[:, :, :, :]

                    def bj(j, rep, tilec):
                        # B_j values: [[0,rep],[1,4]] tiled -> broadcast block
                        return _ap(bb, boff + j * 4, [[0, rep], [1, 4]]) if tilec \
                            else _ap(bb, boff + j * 4, [[1, 4], [0, rep]])

                    nc.vector.tensor_tensor(
                        out=k2[:, :], in0=bj(0, 4, False), in1=bj(1, 4, True), op=MUL)
                    nc.vector.tensor_tensor(
                        out=k3[:, :],
                        in0=_ap(k2[:, :], 0, [[1, 16], [0, 4]]),
                        in1=bj(2, 16, True), op=MUL)
                    nc.vector.tensor_tensor(
                        out=q34[:, :], in0=bj(3, 4, False), in1=bj(4, 4, True), op=MUL)
                    nc.vector.tensor_tensor(
                        out=cnd[:, :],
                        in0=_ap(k3[:, :], 0, [[1, 64], [0, 16]]),
                        in1=_ap(q34[:, :], 0, [[0, 64], [1, 16]]), op=MUL)

                    # transpose 8 128x128 blocks -> 2-bank psum tile
                    pst = ps_t.tile([P, KCH, P], F32, tag="pst")
                    # dummy absorbs the psum-slot-release wait so the first
                    # real transpose carries only the DVE (cond) wait
                    nc.tensor.matmul(out=pst[:2, 0, :2], lhsT=idn[:2, :2],
                                     rhs=idn[:2, :2], is_transpose=True,
                                     start=True, stop=True,
                                     skip_group_check=True)
                    for k in range(KCH):
                        nc.tensor.matmul(
                            out=pst[:, k, :],
                            lhsT=cnd[:, k * P:(k + 1) * P],
                            rhs=idn[:, :],
                            is_transpose=True,
                            start=(k % 4 == 0),
                            stop=(k % 4 == 3),
                        )
                    # copy psum -> condT sbuf (scatter over k, col t*128)
                    for half in range(2):
                        nc.scalar.copy(
                            out=_ap(ctb[:, :, :, :], (half * 4) * NT * P + t * P,
                                    [[NT * P, 4], [1, P]]),
                            in_=_ap(pst[:, :, :], half * 4 * P, [[P, 4], [1, P]]),
                        )

                pmm = ps_mm.tile([M90, NB], F32, tag="pmm")
                for k in range(KCH):
                    nc.tensor.matmul(
                        out=pmm[:, :],
                        lhsT=wsb[:, k, :],
                        rhs=ctb[:, k, :, :],
                        start=(k == 0),
                        stop=(k == KCH - 1),
                    )
                tft = ctbp.tile([M90, NB], F32, tag="tft")
                nc.scalar.copy(out=tft[:, :], in_=pmm[:, :])
                # de-transpose tf^T -> natural [128, t, 90]
                psd = ps_d.tile([P, NT, M90], F32, tag="psd")
                for t in range(NT):
                    nc.tensor.matmul(
                        out=psd[:, t, :],
                        lhsT=tft[:, t * P:(t + 1) * P],
                        rhs=idn[:M90, :M90],
                        is_transpose=True,
                        start=(t == 0),
                        stop=(t == NT - 1),
                    )
                # scatter to tf_big[p, s, :], s = g*NT + t
                nc.scalar.copy(
                    out=_ap(tf_big[:, :, :], g * NT * M90, [[1, NT * M90]]),
                    in_=_ap(psd[:, :, :], 0, [[1, NT * M90]]),
                )

            # ---- stage E: phi/psi ----
            psi_big = bigs.tile([P, S, DIM, 15], F32)
            p15 = bigs.tile([P, S, DIM], F32)
            for j in range(DIM):
                px = powp.tile([P, 16, S], F32, tag="px")
                pq = powp.tile([P, 16, S], F32, tag="pq")
                for (tbl, base) in ((px, xin), (pq, omx)):
                    nc.vector.memset(tbl[:, 0, :], 1.0)
                    nc.vector.tensor_copy(
                        out=tbl[:, 1, :], in_=_ap(base[:, :, :], j, [[DIM, S]]))
                    t1 = tbl[:, :, :]
                    nc.vector.tensor_tensor(
                        out=tbl[:, 2, :], in0=t1[:, 1, :], in1=t1[:, 1, :], op=MUL)
                    nc.vector.tensor_tensor(
                        out=_ap(t1, 3 * S, [[1, 2 * S]]),
                        in0=_ap(t1, S, [[1, 2 * S]]),
                        in1=_ap(t1, 2 * S, [[0, 2], [1, S]]), op=MUL)
                    nc.vector.tensor_tensor(
                        out=_ap(t1, 5 * S, [[1, 4 * S]]),
                        in0=_ap(t1, S, [[1, 4 * S]]),
                        in1=_ap(t1, 4 * S, [[0, 4], [1, S]]), op=MUL)
                    nc.vector.tensor_tensor(
                        out=_ap(t1, 9 * S, [[1, 7 * S]]),
                        in0=_ap(t1, S, [[1, 7 * S]]),
                        in1=_ap(t1, 8 * S, [[0, 7], [1, S]]), op=MUL)
                # v = kappa[a] * x^a * (1-x)^(15-a)   (a-major [16, S])
                v = powp.tile([P, 16, S], F32, tag="v")
                nc.vector.tensor_tensor(
                    out=v[:, :, :],
                    in0=px[:, :, :],
                    in1=_ap(pq[:, :, :], 15 * S, [[-S, 16], [1, S]]), op=MUL)
                nc.vector.tensor_tensor(
                    out=v[:, :, :], in0=v[:, :, :],
                    in1=_ap(kapt[:, :], 0, [[1, 16], [0, S]]), op=MUL)
                # psi[p, s, j, m] = v[m, s] - v[m+1, s]
                nc.vector.tensor_tensor(
                    out=_ap(psi_big[:, :, :, :], j * 15, [[DIM * 15, S], [1, 15]]),
                    in0=_ap(v[:, :, :], 0, [[1, S], [S, 15]]),
                    in1=_ap(v[:, :, :], S, [[1, S], [S, 15]]), op=SUB)
                nc.vector.tensor_copy(
                    out=_ap(p15[:, :, :], j, [[DIM, S]]),
                    in_=_ap(v[:, :, :], 15 * S, [[1, S]]))

            # ---- stage F: combine ----
            ebig = bigs.tile([P, S, DIM, 15], F32)
            nc.vector.tensor_tensor(
                out=ebig[:, :, :, :],
                in0=_ap(tf_big[:, :, :], 0, [[1, S * DIM * 15]]),
                in1=_ap(psi_big[:, :, :, :], 0, [[1, S * DIM * 15]]), op=MUL)
            sred = bigs.tile([P, S, DIM], F32)
            nc.vector.tensor_reduce(
                out=sred[:, :, :],
                in_=_ap(ebig[:, :, :, :], 0, [[15, S * DIM], [1, 15]]),
                op=ADD, axis=mybir.AxisListType.X)
            nc.vector.tensor_tensor(
                out=sred[:, :, :], in0=sred[:, :, :], in1=p15[:, :, :], op=ADD)
            t1 = bigs.tile([P, S, 3], F32)
            nc.vector.tensor_tensor(
                out=t1[:, :, :],
                in0=_ap(sred[:, :, :], 0, [[DIM, S], [2, 3]]),
                in1=_ap(sred[:, :, :], 1, [[DIM, S], [2, 3]]), op=MUL)
            dq = bigs.tile([P, S], F32)
            nc.vector.tensor_tensor(
                out=dq[:, :],
                in0=_ap(t1[:, :, :], 0, [[3, S]]),
                in1=_ap(t1[:, :, :], 1, [[3, S]]), op=MUL)
            nc.vector.tensor_tensor(
                out=dq[:, :], in0=dq[:, :],
                in1=_ap(t1[:, :, :], 2, [[3, S]]), op=MUL)
            nc.sync.dma_start(out=dens_out[:, :], in_=dq[:, :])

    nc.finalize()
    return nc


def _softplus64(v):
    return np.logaddexp(0.0, v)


def _host_w(As):
    cols = []
    for i in range(DIM):
        c = np.cumsum(_softplus64(As[i].astype(np.float64)), axis=1)
        ca = 2.0 * (1.0 / (1.0 + np.exp(-c)) - 0.5)
        cols.append(np.repeat(ca, 4 ** (5 - i), axis=0))
    return np.concatenate(cols, axis=1).astype(np.float32)


def kernel(**inputs):
    x = np.asarray(inputs["x"], dtype=np.float32)
    As = [np.asarray(inputs[f"A{i}"], dtype=np.float32) for i in range(DIM)]

    if "nc" not in _CACHE:
        _CACHE["nc"] = _build_nc()
    nc = _CACHE["nc"]

    w = _host_w(As)
    kapv = (16.0 * np.array([math.comb(15, a) for a in range(16)],
                            dtype=np.float64)).astype(np.float32)[None, :]
    idn = np.eye(P, dtype=np.float32)

    in_maps = []
    for c in range(NCORES):
        xc = x[c * NC:(c + 1) * NC].reshape(P, S, DIM)
        in_maps.append({"xr": xc, "wmat": w, "kap": kapv, "ident": idn})

    res = run_bass_kernel_spmd(nc, in_maps, core_ids=list(range(NCORES)))
    outs = [r["dens"].reshape(NC) for r in res.results]
    return np.concatenate(outs, axis=0)


if __name__ == "__main__":
    rng = np.random.default_rng(0)
    ins = {"x": rng.uniform(0, 1, (N, DIM)).astype(np.float32)}
    for i in range(DIM):
        ins[f"A{i}"] = rng.uniform(0, 1, ((4 ** i), 15)).astype(np.float32)
    out = kernel(**ins)
    print(out.shape, out[:4])


# revision 21
# speedup vs baseline: 1.0458x; 1.0393x over previous
"""Bernstein flow density kernel for 8x TRN2 NeuronCores.

Math (per sample n):
  density(n) = prod_i [ phi_i[n,15] + sum_m tf_i[n,m] * psi_i[n,m] ]
  tf_i = cond_i @ c_alpha_i,  cond_i = B_0 (x) ... (x) B_{i-1}  (row-wise Kron)
Key identity: Bernstein bases sum to 1, so cond_i is a marginal of
cond_5 [N,1024]; all six matmuls merge into ONE:
  tf_all[N, 90] = cond_5 @ W,  W[c, i*15+m] = c_alpha_i[c >> 2*(5-i), m]
psi_i[n,m] = phi_i[n,m] - phi_i[n,m+1] (m=0..14), phi = scaled Bernstein deg-15.

Per core (8192 samples, p-major: local n = p*64 + s):
  1. build deg-3 factor tables B_j [128,(s,j,a)] with vector ops
  2. per s-tile: cond_5 [128,1024] via 4 broadcast-AP tensor_tensor ops
  3. PE-transpose 128x128 blocks -> cond^T chunks; fp32 matmul vs W -> tf^T
  4. PE-transpose tf^T back to natural; build phi/psi; combine + 6-way product
"""

import math
import sys

import numpy as np

sys.path.insert(0, "/opt/trn_rl_repo")

import concourse.bacc as bacc  # noqa: E402
import concourse.bass as bass  # noqa: E402
import concourse.tile as tile  # noqa: E402
from concourse import mybir  # noqa: E402
from concourse.bass_utils import run_bass_kernel_spmd  # noqa: E402

N = 65536
DIM = 6
NCORES = 8
NC = N // NCORES          # 8192 samples per core
P = 128
S = NC // P               # 64 samples per partition
NT = 4                    # s-tiles per matmul group
NG = S // NT              # 16 groups (matmul chunks of 512 samples)
NB = NT * P               # 512 samples per group
CDIM = 1024               # cond_5 width
KCH = CDIM // P           # 8 contraction chunks
M90 = 90                  # 6 dims * 15 coeffs

F32 = mybir.dt.float32
F32R = mybir.dt.float32r
MUL = mybir.AluOpType.mult
ADD = mybir.AluOpType.add
SUB = mybir.AluOpType.subtract

_CACHE = {}


def _ap(a, off_elems, dims):
    """AP over slice a with replaced free dims; dims = [[step,count],...]."""
    return bass.AP(tensor=a.tensor, offset=a.offset + off_elems, ap=[a.ap[0]] + dims)


def _build_nc(mm_dtype=F32):
    nc = bacc.Bacc(target_bir_lowering=False, trn_type="TRN2")

    xr = nc.dram_tensor("xr", [P, S, DIM], F32, kind="ExternalInput")
    wmat = nc.dram_tensor("wmat", [CDIM, M90], F32, kind="ExternalInput")
    kap = nc.dram_tensor("kap", [1, 16], F32, kind="ExternalInput")
    ident = nc.dram_tensor("ident", [P, P], F32, kind="ExternalInput")
    dens_out = nc.dram_tensor("dens", [P, S], F32, kind="ExternalOutput")

    with tile.TileContext(nc) as tc:
        with (
            tc.tile_pool(name="singles", bufs=1) as singles,
            tc.tile_pool(name="bigs", bufs=1) as bigs,
            tc.tile_pool(name="cond", bufs=3) as condp,
            tc.tile_pool(name="ctb", bufs=3) as ctbp,
            tc.tile_pool(name="pows", bufs=2) as powp,
            tc.tile_pool(name="ps_t", bufs=2, space="PSUM") as ps_t,
            tc.tile_pool(name="ps_mm", bufs=2, space="PSUM") as ps_mm,
            tc.tile_pool(name="ps_d", bufs=1, space="PSUM") as ps_d,
            tc.tile_pool(name="ps_x", bufs=1, space="PSUM") as ps_x,
        ):
            # ---- constants / inputs ----
            xin = singles.tile([P, S, DIM], F32)
            nc.sync.dma_start(out=xin[:, :, :], in_=xr[:, :, :])
            wsb = singles.tile([P, KCH, M90], F32)
            nc.sync.dma_start(
                out=wsb[:, :, :],
                in_=bass.AP(tensor=wmat[:, :].tensor, offset=0,
                            ap=[[M90, P], [P * M90, KCH], [1, M90]]),
            )
            idn = singles.tile([P, P], F32)
            nc.sync.dma_start(out=idn[:, :], in_=ident[:, :])
            kapt = singles.tile([P, 16], F32)
            nc.sync.dma_start(
                out=kapt[:, :],
                in_=bass.AP(tensor=kap[:, :].tensor, offset=0, ap=[[0, P], [1, 16]]),
            )

            # PE "pre-observe" dummies: walrus fp32 fused matmul (LDW+MM)
            # tolerates only one sync wait, so make the PE observe the DMA
            # semaphores up front via tiny throwaway transposes.
            scr = ps_x.tile([2, 2], F32)
            nc.tensor.matmul(out=scr[:, :], lhsT=idn[:2, :2], rhs=idn[:2, :2],
                             is_transpose=True, start=True, stop=True,
                             skip_group_check=True)
            nc.tensor.matmul(out=scr[:, :], lhsT=wsb[:2, 0, :2], rhs=idn[:2, :2],
                             is_transpose=True, start=True, stop=True,
                             skip_group_check=True)

            xa = xin[:, :, :]

            # ---- stage A: powers of x, 1-x ----
            FD6 = S * DIM
            omx = singles.tile([P, S, DIM], F32)
            x2 = singles.tile([P, S, DIM], F32)
            x3 = singles.tile([P, S, DIM], F32)
            omx2 = singles.tile([P, S, DIM], F32)
            omx3 = singles.tile([P, S, DIM], F32)
            # omx = (x * -1) + 1
            nc.vector.tensor_scalar(
                out=omx[:, :, :], in0=xa, scalar1=-1.0, scalar2=1.0, op0=MUL, op1=ADD
            )
            nc.vector.tensor_tensor(out=x2[:, :, :], in0=xa, in1=xa, op=MUL)
            nc.vector.tensor_tensor(
                out=omx2[:, :, :], in0=omx[:, :, :], in1=omx[:, :, :], op=MUL
            )
            nc.vector.tensor_tensor(out=x3[:, :, :], in0=x2[:, :, :], in1=xa, op=MUL)
            nc.vector.tensor_tensor(
                out=omx3[:, :, :], in0=omx2[:, :, :], in1=omx[:, :, :], op=MUL
            )

            # ---- stage B: deg-3 tables Bbig[p, s, j, a]  j=0..4 ----
            NJ = 5
            Bbig = singles.tile([P, S, NJ, 4], F32)
            for (a, src, scl, other) in (
                (0, omx3, None, None),
                (1, xin, 3.0, omx2),
                (2, x2, 3.0, omx),
                (3, x3, None, None),
            ):
                src_ap = _ap(src[:, :, :], 0, [[DIM, S], [1, NJ]])
                out_ap = _ap(Bbig[:, :, :, :], a, [[4 * NJ, S], [4, NJ]])
                if scl is None:
                    nc.vector.tensor_copy(out=out_ap, in_=src_ap)
                else:
                    nc.vector.scalar_tensor_tensor(
                        out=out_ap, in0=src_ap, scalar=scl,
                        in1=_ap(other[:, :, :], 0, [[DIM, S], [1, NJ]]),
                        op0=MUL, op1=MUL,
                    )

            # ---- stage C+D: cond tiles, transpose, matmul per group ----
            tf_big = bigs.tile([P, S, M90], F32)   # natural-layout tf
            psi_big = bigs.tile([P, S, DIM, 15], F32)
            p15 = bigs.tile([P, S, DIM], F32)
            ebig = bigs.tile([P, S, DIM, 15], F32)
            sred = bigs.tile([P, S, DIM], F32)

            def emit_phipsi(j):
                px = powp.tile([P, 16, S], F32, tag="px")
                pq = powp.tile([P, 16, S], F32, tag="pq")
                for (tbl, base) in ((px, xin), (pq, omx)):
                    nc.vector.memset(tbl[:, 0, :], 1.0)
                    nc.vector.tensor_copy(
                        out=tbl[:, 1, :], in_=_ap(base[:, :, :], j, [[DIM, S]]))
                    t1 = tbl[:, :, :]
                    nc.vector.tensor_tensor(
                        out=tbl[:, 2, :], in0=t1[:, 1, :], in1=t1[:, 1, :], op=MUL)
                    nc.vector.tensor_tensor(
                        out=_ap(t1, 3 * S, [[1, 2 * S]]),
                        in0=_ap(t1, S, [[1, 2 * S]]),
                        in1=_ap(t1, 2 * S, [[0, 2], [1, S]]), op=MUL)
                    nc.any.tensor_tensor(
                        out=_ap(t1, 5 * S, [[1, 4 * S]]),
                        in0=_ap(t1, S, [[1, 4 * S]]),
                        in1=_ap(t1, 4 * S, [[0, 4], [1, S]]), op=MUL)
                    nc.any.tensor_tensor(
                        out=_ap(t1, 9 * S, [[1, 7 * S]]),
                        in0=_ap(t1, S, [[1, 7 * S]]),
                        in1=_ap(t1, 8 * S, [[0, 7], [1, S]]), op=MUL)
                v = powp.tile([P, 16, S], F32, tag="v")
                nc.any.tensor_tensor(
                    out=v[:, :, :],
                    in0=px[:, :, :],
                    in1=_ap(pq[:, :, :], 15 * S, [[-S, 16], [1, S]]), op=MUL)
                nc.any.tensor_tensor(
                    out=v[:, :, :], in0=v[:, :, :],
                    in1=_ap(kapt[:, :], 0, [[1, 16], [0, S]]), op=MUL)
                nc.any.tensor_tensor(
                    out=_ap(psi_big[:, :, :, :], j * 15, [[DIM * 15, S], [1, 15]]),
                    in0=_ap(v[:, :, :], 0, [[1, S], [S, 15]]),
                    in1=_ap(v[:, :, :], S, [[1, S], [S, 15]]), op=SUB)
                nc.vector.tensor_copy(
                    out=_ap(p15[:, :, :], j, [[DIM, S]]),
                    in_=_ap(v[:, :, :], 15 * S, [[1, S]]))

            def emit_combine(s0, s1):
                ns = s1 - s0
                nc.any.tensor_tensor(
                    out=_ap(ebig[:, :, :, :], s0 * M90, [[1, ns * M90]]),
                    in0=_ap(tf_big[:, :, :], s0 * M90, [[1, ns * M90]]),
                    in1=_ap(psi_big[:, :, :, :], s0 * M90, [[1, ns * M90]]), op=MUL)
                nc.vector.tensor_reduce(
                    out=_ap(sred[:, :, :], s0 * DIM, [[1, ns * DIM]]),
                    in_=_ap(ebig[:, :, :, :], s0 * M90, [[15, ns * DIM], [1, 15]]),
                    op=ADD, axis=mybir.AxisListType.X)

            for g in range(NG):
                ctb = ctbp.tile([P, KCH, NT, P], F32, tag="ctb")
                for t in range(NT):
                    s = g * NT + t
                    cnd = condp.tile([P, CDIM], F32, tag="cond")
                    k2 = condp.tile([P, 16], F32, tag="k2")
                    k3 = condp.tile([P, 64], F32, tag="k3")
                    q34 = condp.tile([P, 16], F32, tag="q34")
                    boff = s * NJ * 4
                    bb = Bbig[:, :, :, :]

                    def bj(j, rep, tilec):
                        # B_j values: [[0,rep],[1,4]] tiled -> broadcast block
                        return _ap(bb, boff + j * 4, [[0, rep], [1, 4]]) if tilec \
                            else _ap(bb, boff + j * 4, [[1, 4], [0, rep]])

                    nc.vector.tensor_tensor(
                        out=k2[:, :], in0=bj(0, 4, False), in1=bj(1, 4, True), op=MUL)
                    nc.vector.tensor_tensor(
                        out=k3[:, :],
                        in0=_ap(k2[:, :], 0, [[1, 16], [0, 4]]),
                        in1=bj(2, 16, True), op=MUL)
                    nc.vector.tensor_tensor(
                        out=q34[:, :], in0=bj(3, 4, False), in1=bj(4, 4, True), op=MUL)
                    nc.any.tensor_tensor(
                        out=cnd[:, :],
                        in0=_ap(k3[:, :], 0, [[1, 64], [0, 16]]),
                        in1=_ap(q34[:, :], 0, [[0, 64], [1, 16]]), op=MUL)

                    # transpose 8 128x128 blocks -> 2-bank psum tile
                    pst = ps_t.tile([P, KCH, P], F32, tag="pst")
                    # dummy absorbs the psum-slot-release wait so the first
                    # real transpose carries only the DVE (cond) wait
                    nc.tensor.matmul(out=pst[:2, 0, :2], lhsT=idn[:2, :2],
                                     rhs=idn[:2, :2], is_transpose=True,
                                     start=True, stop=True,
                                     skip_group_check=True)
                    for k in range(KCH):
                        nc.tensor.matmul(
                            out=pst[:, k, :],
                            lhsT=cnd[:, k * P:(k + 1) * P],
                            rhs=idn[:, :],
                            is_transpose=True,
                            start=(k % 4 == 0),
                            stop=(k % 4 == 3),
                        )
                    # copy psum -> condT sbuf (scatter over k, col t*128)
                    for half in range(2):
                        nc.scalar.copy(
                            out=_ap(ctb[:, :, :, :], (half * 4) * NT * P + t * P,
                                    [[NT * P, 4], [1, P]]),
                            in_=_ap(pst[:, :, :], half * 4 * P, [[P, 4], [1, P]]),
                        )

                pmm = ps_mm.tile([M90, NB], F32, tag="pmm")
                for k in range(KCH):
                    nc.tensor.matmul(
                        out=pmm[:, :],
                        lhsT=wsb[:, k, :],
                        rhs=ctb[:, k, :, :],
                        start=(k == 0),
                        stop=(k == KCH - 1),
                    )
                tft = ctbp.tile([M90, NB], F32, tag="tft")
                nc.scalar.copy(out=tft[:, :], in_=pmm[:, :])
                # de-transpose tf^T -> natural [128, t, 90]
                psd = ps_d.tile([P, NT, M90], F32, tag="psd")
                for t in range(NT):
                    nc.tensor.matmul(
                        out=psd[:, t, :],
                        lhsT=tft[:, t * P:(t + 1) * P],
                        rhs=idn[:M90, :M90],
                        is_transpose=True,
                        start=(t == 0),
                        stop=(t == NT - 1),
                    )
                # scatter to tf_big[p, s, :], s = g*NT + t
                nc.scalar.copy(
                    out=_ap(tf_big[:, :, :], g * NT * M90, [[1, NT * M90]]),
                    in_=_ap(psd[:, :, :], 0, [[1, NT * M90]]),
                )
                if g < DIM:
                    emit_phipsi(g)
                elif g in (9, 12, 15):
                    # psi complete after group 5; combine finished s-ranges
                    done = {9: (0, 32), 12: (32, 48), 15: (48, 64)}[g]
                    emit_combine(*done)

            # ---- stage F: final combine tail ----
            nc.vector.tensor_tensor(
                out=sred[:, :, :], in0=sred[:, :, :], in1=p15[:, :, :], op=ADD)
            t1 = bigs.tile([P, S, 3], F32)
            nc.vector.tensor_tensor(
                out=t1[:, :, :],
                in0=_ap(sred[:, :, :], 0, [[DIM, S], [2, 3]]),
                in1=_ap(sred[:, :, :], 1, [[DIM, S], [2, 3]]), op=MUL)
            dq = bigs.tile([P, S], F32)
            nc.vector.tensor_tensor(
                out=dq[:, :],
                in0=_ap(t1[:, :, :], 0, [[3, S]]),
                in1=_ap(t1[:, :, :], 1, [[3, S]]), op=MUL)
            nc.vector.tensor_tensor(
                out=dq[:, :], in0=dq[:, :],
                in1=_ap(t1[:, :, :], 2, [[3, S]]), op=MUL)
            nc.sync.dma_start(out=dens_out[:, :], in_=dq[:, :])

    nc.finalize()
    return nc


def _softplus64(v):
    return np.logaddexp(0.0, v)


def _host_w(As):
    cols = []
    for i in range(DIM):
        c = np.cumsum(_softplus64(As[i].astype(np.float64)), axis=1)
        ca = 2.0 * (1.0 / (1.0 + np.exp(-c)) - 0.5)
        cols.append(np.repeat(ca, 4 ** (5 - i), axis=0))
    return np.concatenate(cols, axis=1).astype(np.float32)


def kernel(**inputs):
    x = np.asarray(inputs["x"], dtype=np.float32)
    As = [np.asarray(inputs[f"A{i}"], dtype=np.float32) for i in range(DIM)]

    if "nc" not in _CACHE:
        _CACHE["nc"] = _build_nc()
    nc = _CACHE["nc"]

    w = _host_w(As)
    kapv = (16.0 * np.array([math.comb(15, a) for a in range(16)],
                            dtype=np.float64)).astype(np.float32)[None, :]
    idn = np.eye(P, dtype=np.float32)

    in_maps = []
    for c in range(NCORES):
        xc = x[c * NC:(c + 1) * NC].reshape(P, S, DIM)
        in_maps.append({"xr": xc, "wmat": w, "kap": kapv, "ident": idn})

    res = run_bass_kernel_spmd(nc, in_maps, core_ids=list(range(NCORES)))
    outs = [r["dens"].reshape(NC) for r in res.results]
    return np.concatenate(outs, axis=0)


if __name__ == "__main__":
    rng = np.random.default_rng(0)
    ins = {"x": rng.uniform(0, 1, (N, DIM)).astype(np.float32)}
    for i in range(DIM):
        ins[f"A{i}"] = rng.uniform(0, 1, ((4 ** i), 15)).astype(np.float32)
    out = kernel(**ins)
    print(out.shape, out[:4])


# revision 23
# speedup vs baseline: 1.3984x; 1.3372x over previous
"""Bernstein flow density kernel for 8x TRN2 NeuronCores.

Math (per sample n):
  density(n) = prod_i [ phi_i[n,15] + sum_m tf_i[n,m] * psi_i[n,m] ]
  tf_i = cond_i @ c_alpha_i,  cond_i = B_0 (x) ... (x) B_{i-1}  (row-wise Kron)
Key identity: Bernstein bases sum to 1, so cond_i is a marginal of
cond_5 [N,1024]; all six matmuls merge into ONE:
  tf_all[N, 90] = cond_5 @ W,  W[c, i*15+m] = c_alpha_i[c >> 2*(5-i), m]
psi_i[n,m] = phi_i[n,m] - phi_i[n,m+1] (m=0..14), phi = scaled Bernstein deg-15.

Per core (8192 samples, p-major: local n = p*64 + s):
  1. build deg-3 factor tables B_j [128,(s,j,a)] with vector ops
  2. per s-tile: cond_5 [128,1024] via 4 broadcast-AP tensor_tensor ops
  3. PE-transpose 128x128 blocks -> cond^T chunks; fp32 matmul vs W -> tf^T
  4. PE-transpose tf^T back to natural; build phi/psi; combine + 6-way product
"""

import math
import sys

import numpy as np

sys.path.insert(0, "/opt/trn_rl_repo")

import concourse.bacc as bacc  # noqa: E402
import concourse.bass as bass  # noqa: E402
import concourse.tile as tile  # noqa: E402
from concourse import mybir  # noqa: E402
from concourse.bass_utils import run_bass_kernel_spmd  # noqa: E402

N = 65536
DIM = 6
NCORES = 8
NC = N // NCORES          # 8192 samples per core
P = 128
S = NC // P               # 64 samples per partition
NT = 4                    # s-tiles per matmul group
NG = S // NT              # 16 groups (matmul chunks of 512 samples)
NB = NT * P               # 512 samples per group
CDIM = 1024               # cond_5 width
KCH = CDIM // P           # 8 contraction chunks
M90 = 90                  # 6 dims * 15 coeffs

F32 = mybir.dt.float32
F32R = mybir.dt.float32r
MUL = mybir.AluOpType.mult
ADD = mybir.AluOpType.add
SUB = mybir.AluOpType.subtract

_CACHE = {}


def _ap(a, off_elems, dims):
    """AP over slice a with replaced free dims; dims = [[step,count],...]."""
    return bass.AP(tensor=a.tensor, offset=a.offset + off_elems, ap=[a.ap[0]] + dims)


def _build_nc(mm_dtype=F32):
    nc = bacc.Bacc(target_bir_lowering=False, trn_type="TRN2")

    xr = nc.dram_tensor("xr", [P, S, DIM], F32, kind="ExternalInput")
    wmat = nc.dram_tensor("wmat", [CDIM, M90], F32, kind="ExternalInput")
    kap = nc.dram_tensor("kap", [1, 16], F32, kind="ExternalInput")
    ident = nc.dram_tensor("ident", [P, P], F32, kind="ExternalInput")
    dens_out = nc.dram_tensor("dens", [P, S], F32, kind="ExternalOutput")

    with tile.TileContext(nc) as tc:
        with (
            tc.tile_pool(name="singles", bufs=1) as singles,
            tc.tile_pool(name="bigs", bufs=1) as bigs,
            tc.tile_pool(name="cond", bufs=3) as condp,
            tc.tile_pool(name="ctb", bufs=3) as ctbp,
            tc.tile_pool(name="pows", bufs=2) as powp,
            tc.tile_pool(name="ps_t", bufs=2, space="PSUM") as ps_t,
            tc.tile_pool(name="ps_mm", bufs=2, space="PSUM") as ps_mm,
            tc.tile_pool(name="ps_x", bufs=1, space="PSUM") as ps_x,
        ):
            # ---- constants / inputs ----
            xin = singles.tile([P, S, DIM], F32)
            nc.sync.dma_start(out=xin[:, :, :], in_=xr[:, :, :])
            wsb = singles.tile([P, KCH, M90], F32)
            nc.sync.dma_start(
                out=wsb[:, :, :],
                in_=bass.AP(tensor=wmat[:, :].tensor, offset=0,
                            ap=[[M90, P], [P * M90, KCH], [1, M90]]),
            )
            idn = singles.tile([P, P], F32)
            nc.sync.dma_start(out=idn[:, :], in_=ident[:, :])
            kapt = singles.tile([P, 16], F32)
            nc.sync.dma_start(
                out=kapt[:, :],
                in_=bass.AP(tensor=kap[:, :].tensor, offset=0, ap=[[0, P], [1, 16]]),
            )

            # PE "pre-observe" dummies: walrus fp32 fused matmul (LDW+MM)
            # tolerates only one sync wait, so make the PE observe the DMA
            # semaphores up front via tiny throwaway transposes.
            scr = ps_x.tile([2, 2], F32)
            nc.tensor.matmul(out=scr[:, :], lhsT=idn[:2, :2], rhs=idn[:2, :2],
                             is_transpose=True, start=True, stop=True,
                             skip_group_check=True)
            nc.tensor.matmul(out=scr[:, :], lhsT=wsb[:2, 0, :2], rhs=idn[:2, :2],
                             is_transpose=True, start=True, stop=True,
                             skip_group_check=True)

            xa = xin[:, :, :]

            # ---- stage A: powers of x, 1-x ----
            FD6 = S * DIM
            omx = singles.tile([P, S, DIM], F32)
            x2 = singles.tile([P, S, DIM], F32)
            x3 = singles.tile([P, S, DIM], F32)
            omx2 = singles.tile([P, S, DIM], F32)
            omx3 = singles.tile([P, S, DIM], F32)
            # omx = (x * -1) + 1
            nc.vector.tensor_scalar(
                out=omx[:, :, :], in0=xa, scalar1=-1.0, scalar2=1.0, op0=MUL, op1=ADD
            )
            nc.vector.tensor_tensor(out=x2[:, :, :], in0=xa, in1=xa, op=MUL)
            nc.vector.tensor_tensor(
                out=omx2[:, :, :], in0=omx[:, :, :], in1=omx[:, :, :], op=MUL
            )
            nc.vector.tensor_tensor(out=x3[:, :, :], in0=x2[:, :, :], in1=xa, op=MUL)
            nc.vector.tensor_tensor(
                out=omx3[:, :, :], in0=omx2[:, :, :], in1=omx[:, :, :], op=MUL
            )

            # ---- stage B: deg-3 tables Bbig[p, s, j, a]  j=0..4 ----
            NJ = 5
            Bbig = singles.tile([P, S, NJ, 4], F32)
            for (a, src, scl, other) in (
                (0, omx3, None, None),
                (1, xin, 3.0, omx2),
                (2, x2, 3.0, omx),
                (3, x3, None, None),
            ):
                src_ap = _ap(src[:, :, :], 0, [[DIM, S], [1, NJ]])
                out_ap = _ap(Bbig[:, :, :, :], a, [[4 * NJ, S], [4, NJ]])
                if scl is None:
                    nc.vector.tensor_copy(out=out_ap, in_=src_ap)
                else:
                    nc.vector.scalar_tensor_tensor(
                        out=out_ap, in0=src_ap, scalar=scl,
                        in1=_ap(other[:, :, :], 0, [[DIM, S], [1, NJ]]),
                        op0=MUL, op1=MUL,
                    )

            # ---- stage C+D: cond tiles, transpose, matmul per group ----
            tf_big = bigs.tile([P, S, M90], F32)   # natural-layout tf
            psi_big = bigs.tile([P, S, DIM, 15], F32)
            p15 = bigs.tile([P, S, DIM], F32)
            ebig = bigs.tile([P, S, DIM, 15], F32)
            sred = bigs.tile([P, S, DIM], F32)

            def emit_phipsi(j):
                px = powp.tile([P, 16, S], F32, tag="px")
                pq = powp.tile([P, 16, S], F32, tag="pq")
                for (tbl, base) in ((px, xin), (pq, omx)):
                    nc.vector.memset(tbl[:, 0, :], 1.0)
                    nc.vector.tensor_copy(
                        out=tbl[:, 1, :], in_=_ap(base[:, :, :], j, [[DIM, S]]))
                    t1 = tbl[:, :, :]
                    nc.vector.tensor_tensor(
                        out=tbl[:, 2, :], in0=t1[:, 1, :], in1=t1[:, 1, :], op=MUL)
                    nc.vector.tensor_tensor(
                        out=_ap(t1, 3 * S, [[1, 2 * S]]),
                        in0=_ap(t1, S, [[1, 2 * S]]),
                        in1=_ap(t1, 2 * S, [[0, 2], [1, S]]), op=MUL)
                    nc.any.tensor_tensor(
                        out=_ap(t1, 5 * S, [[1, 4 * S]]),
                        in0=_ap(t1, S, [[1, 4 * S]]),
                        in1=_ap(t1, 4 * S, [[0, 4], [1, S]]), op=MUL)
                    nc.any.tensor_tensor(
                        out=_ap(t1, 9 * S, [[1, 7 * S]]),
                        in0=_ap(t1, S, [[1, 7 * S]]),
                        in1=_ap(t1, 8 * S, [[0, 7], [1, S]]), op=MUL)
                v = powp.tile([P, 16, S], F32, tag="v")
                nc.any.tensor_tensor(
                    out=v[:, :, :],
                    in0=px[:, :, :],
                    in1=_ap(pq[:, :, :], 15 * S, [[-S, 16], [1, S]]), op=MUL)
                nc.any.tensor_tensor(
                    out=v[:, :, :], in0=v[:, :, :],
                    in1=_ap(kapt[:, :], 0, [[1, 16], [0, S]]), op=MUL)
                nc.any.tensor_tensor(
                    out=_ap(psi_big[:, :, :, :], j * 15, [[DIM * 15, S], [1, 15]]),
                    in0=_ap(v[:, :, :], 0, [[1, S], [S, 15]]),
                    in1=_ap(v[:, :, :], S, [[1, S], [S, 15]]), op=SUB)
                nc.vector.tensor_copy(
                    out=_ap(p15[:, :, :], j, [[DIM, S]]),
                    in_=_ap(v[:, :, :], 15 * S, [[1, S]]))

            def emit_combine(s0, s1):
                ns = s1 - s0
                nc.any.tensor_tensor(
                    out=_ap(ebig[:, :, :, :], s0 * M90, [[1, ns * M90]]),
                    in0=_ap(tf_big[:, :, :], s0 * M90, [[1, ns * M90]]),
                    in1=_ap(psi_big[:, :, :, :], s0 * M90, [[1, ns * M90]]), op=MUL)
                nc.vector.tensor_reduce(
                    out=_ap(sred[:, :, :], s0 * DIM, [[1, ns * DIM]]),
                    in_=_ap(ebig[:, :, :, :], s0 * M90, [[15, ns * DIM], [1, 15]]),
                    op=ADD, axis=mybir.AxisListType.X)

            for g in range(NG):
                ctb = ctbp.tile([P, KCH, NT, P], F32, tag="ctb")
                bb = Bbig[:, :, :, :]
                gb = g * NT * NJ * 4   # B-table offset of this group's tiles
                TS = NJ * 4            # per-tile stride in Bbig cols
                k2g = condp.tile([P, NT, 16], F32, tag="k2")
                k3g = condp.tile([P, NT, 64], F32, tag="k3")
                q34g = condp.tile([P, NT, 16], F32, tag="q34")
                nc.vector.tensor_tensor(
                    out=k2g[:, :, :],
                    in0=_ap(bb, gb + 0, [[TS, NT], [1, 4], [0, 4]]),
                    in1=_ap(bb, gb + 4, [[TS, NT], [0, 4], [1, 4]]), op=MUL)
                nc.vector.tensor_tensor(
                    out=k3g[:, :, :],
                    in0=_ap(k2g[:, :, :], 0, [[16, NT], [1, 16], [0, 4]]),
                    in1=_ap(bb, gb + 8, [[TS, NT], [0, 16], [1, 4]]), op=MUL)
                nc.vector.tensor_tensor(
                    out=q34g[:, :, :],
                    in0=_ap(bb, gb + 12, [[TS, NT], [1, 4], [0, 4]]),
                    in1=_ap(bb, gb + 16, [[TS, NT], [0, 4], [1, 4]]), op=MUL)
                for t in range(NT):
                    cnd = condp.tile([P, CDIM], F32, tag="cond")
                    nc.any.tensor_tensor(
                        out=cnd[:, :],
                        in0=_ap(k3g[:, :, :], t * 64, [[1, 64], [0, 16]]),
                        in1=_ap(q34g[:, :, :], t * 16, [[0, 64], [1, 16]]), op=MUL)

                    # transpose 8 128x128 blocks -> 2-bank psum tile
                    pst = ps_t.tile([P, KCH, P], F32, tag="pst")
                    # dummy absorbs the psum-slot-release wait so the first
                    # real transpose carries only the DVE (cond) wait
                    nc.tensor.matmul(out=pst[:2, 0, :2], lhsT=idn[:2, :2],
                                     rhs=idn[:2, :2], is_transpose=True,
                                     start=True, stop=True,
                                     skip_group_check=True)
                    for k in range(KCH):
                        nc.tensor.matmul(
                            out=pst[:, k, :],
                            lhsT=cnd[:, k * P:(k + 1) * P],
                            rhs=idn[:, :],
                            is_transpose=True,
                            start=(k % 4 == 0),
                            stop=(k % 4 == 3),
                        )
                    # copy psum -> condT sbuf (scatter over k, col t*128)
                    for half in range(2):
                        nc.scalar.copy(
                            out=_ap(ctb[:, :, :, :], (half * 4) * NT * P + t * P,
                                    [[NT * P, 4], [1, P]]),
                            in_=_ap(pst[:, :, :], half * 4 * P, [[P, 4], [1, P]]),
                        )

                # tf natural directly: stationary cond^T, moving W (90 cols)
                for t in range(NT):
                    pmm = ps_mm.tile([P, M90], F32, tag="pmm")
                    for k in range(KCH):
                        nc.tensor.matmul(
                            out=pmm[:, :],
                            lhsT=ctb[:, k, t, :],
                            rhs=wsb[:, k, :],
                            start=(k == 0),
                            stop=(k == KCH - 1),
                        )
                    nc.scalar.copy(
                        out=_ap(tf_big[:, :, :], (g * NT + t) * M90, [[1, M90]]),
                        in_=pmm[:, :],
                    )
                if g < DIM:
                    emit_phipsi(g)
                elif g in (9, 12, 15):
                    # psi complete after group 5; combine finished s-ranges
                    done = {9: (0, 32), 12: (32, 48), 15: (48, 64)}[g]
                    emit_combine(*done)

            # ---- stage F: final combine tail ----
            nc.vector.tensor_tensor(
                out=sred[:, :, :], in0=sred[:, :, :], in1=p15[:, :, :], op=ADD)
            t1 = bigs.tile([P, S, 3], F32)
            nc.vector.tensor_tensor(
                out=t1[:, :, :],
                in0=_ap(sred[:, :, :], 0, [[DIM, S], [2, 3]]),
                in1=_ap(sred[:, :, :], 1, [[DIM, S], [2, 3]]), op=MUL)
            dq = bigs.tile([P, S], F32)
            nc.vector.tensor_tensor(
                out=dq[:, :],
                in0=_ap(t1[:, :, :], 0, [[3, S]]),
                in1=_ap(t1[:, :, :], 1, [[3, S]]), op=MUL)
            nc.vector.tensor_tensor(
                out=dq[:, :], in0=dq[:, :],
                in1=_ap(t1[:, :, :], 2, [[3, S]]), op=MUL)
            nc.sync.dma_start(out=dens_out[:, :], in_=dq[:, :])

    nc.finalize()
    return nc


def _softplus64(v):
    return np.logaddexp(0.0, v)


def _host_w(As):
    cols = []
    for i in range(DIM):
        c = np.cumsum(_softplus64(As[i].astype(np.float64)), axis=1)
        ca = 2.0 * (1.0 / (1.0 + np.exp(-c)) - 0.5)
        cols.append(np.repeat(ca, 4 ** (5 - i), axis=0))
    return np.concatenate(cols, axis=1).astype(np.float32)


def kernel(**inputs):
    x = np.asarray(inputs["x"], dtype=np.float32)
    As = [np.asarray(inputs[f"A{i}"], dtype=np.float32) for i in range(DIM)]

    if "nc" not in _CACHE:
        _CACHE["nc"] = _build_nc()
    nc = _CACHE["nc"]

    w = _host_w(As)
    kapv = (16.0 * np.array([math.comb(15, a) for a in range(16)],
                            dtype=np.float64)).astype(np.float32)[None, :]
    idn = np.eye(P, dtype=np.float32)

    in_maps = []
    for c in range(NCORES):
        xc = x[c * NC:(c + 1) * NC].reshape(P, S, DIM)
        in_maps.append({"xr": xc, "wmat": w, "kap": kapv, "ident": idn})

    res = run_bass_kernel_spmd(nc, in_maps, core_ids=list(range(NCORES)))
    outs = [r["dens"].reshape(NC) for r in res.results]
    return np.concatenate(outs, axis=0)


if __name__ == "__main__":
    rng = np.random.default_rng(0)
    ins = {"x": rng.uniform(0, 1, (N, DIM)).astype(np.float32)}
    for i in range(DIM):
        ins[f"A{i}"] = rng.uniform(0, 1, ((4 ** i), 15)).astype(np.float32)
    out = kernel(**ins)
    print(out.shape, out[:4])


# revision 24
# speedup vs baseline: 1.4684x; 1.0501x over previous
"""Bernstein flow density kernel for 8x TRN2 NeuronCores.

Math (per sample n):
  density(n) = prod_i [ phi_i[n,15] + sum_m tf_i[n,m] * psi_i[n,m] ]
  tf_i = cond_i @ c_alpha_i,  cond_i = B_0 (x) ... (x) B_{i-1}  (row-wise Kron)
Key identity: Bernstein bases sum to 1, so cond_i is a marginal of
cond_5 [N,1024]; all six matmuls merge into ONE:
  tf_all[N, 90] = cond_5 @ W,  W[c, i*15+m] = c_alpha_i[c >> 2*(5-i), m]
psi_i[n,m] = phi_i[n,m] - phi_i[n,m+1] (m=0..14), phi = scaled Bernstein deg-15.

Per core (8192 samples, p-major: local n = p*64 + s):
  1. build deg-3 factor tables B_j [128,(s,j,a)] with vector ops
  2. per s-tile: cond_5 [128,1024] via 4 broadcast-AP tensor_tensor ops
  3. PE-transpose 128x128 blocks -> cond^T chunks; fp32 matmul vs W -> tf^T
  4. PE-transpose tf^T back to natural; build phi/psi; combine + 6-way product
"""

import math
import sys

import numpy as np

sys.path.insert(0, "/opt/trn_rl_repo")

import concourse.bacc as bacc  # noqa: E402
import concourse.bass as bass  # noqa: E402
import concourse.tile as tile  # noqa: E402
from concourse import mybir  # noqa: E402
from concourse.bass_utils import run_bass_kernel_spmd  # noqa: E402

N = 65536
DIM = 6
NCORES = 8
NC = N // NCORES          # 8192 samples per core
P = 128
S = NC // P               # 64 samples per partition
NT = 4                    # s-tiles per matmul group
NG = S // NT              # 16 groups (matmul chunks of 512 samples)
NB = NT * P               # 512 samples per group
CDIM = 1024               # cond_5 width
KCH = CDIM // P           # 8 contraction chunks
M90 = 90                  # 6 dims * 15 coeffs

F32 = mybir.dt.float32
F32R = mybir.dt.float32r
MUL = mybir.AluOpType.mult
ADD = mybir.AluOpType.add
SUB = mybir.AluOpType.subtract

_CACHE = {}


def _ap(a, off_elems, dims):
    """AP over slice a with replaced free dims; dims = [[step,count],...]."""
    return bass.AP(tensor=a.tensor, offset=a.offset + off_elems, ap=[a.ap[0]] + dims)


def _build_nc(mm_dtype=F32):
    nc = bacc.Bacc(target_bir_lowering=False, trn_type="TRN2")

    xr = nc.dram_tensor("xr", [P, S, DIM], F32, kind="ExternalInput")
    wmat = nc.dram_tensor("wmat", [CDIM, M90], F32, kind="ExternalInput")
    kap = nc.dram_tensor("kap", [1, 16], F32, kind="ExternalInput")
    ident = nc.dram_tensor("ident", [P, P], F32, kind="ExternalInput")
    dens_out = nc.dram_tensor("dens", [P, S], F32, kind="ExternalOutput")

    with tile.TileContext(nc) as tc:
        with (
            tc.tile_pool(name="singles", bufs=1) as singles,
            tc.tile_pool(name="bigs", bufs=1) as bigs,
            tc.tile_pool(name="cond", bufs=3) as condp,
            tc.tile_pool(name="ctb", bufs=3) as ctbp,
            tc.tile_pool(name="pows", bufs=2) as powp,
            tc.tile_pool(name="ps_t", bufs=2, space="PSUM") as ps_t,
            tc.tile_pool(name="ps_mm", bufs=2, space="PSUM") as ps_mm,
            tc.tile_pool(name="ps_x", bufs=1, space="PSUM") as ps_x,
        ):
            # ---- constants / inputs ----
            xin = singles.tile([P, S, DIM], F32)
            nc.sync.dma_start(out=xin[:, :, :], in_=xr[:, :, :])
            wsb = singles.tile([P, KCH, M90], F32)
            nc.sync.dma_start(
                out=wsb[:, :, :],
                in_=bass.AP(tensor=wmat[:, :].tensor, offset=0,
                            ap=[[M90, P], [P * M90, KCH], [1, M90]]),
            )
            idn = singles.tile([P, P], F32)
            nc.sync.dma_start(out=idn[:, :], in_=ident[:, :])
            kapt = singles.tile([P, 16], F32)
            nc.sync.dma_start(
                out=kapt[:, :],
                in_=bass.AP(tensor=kap[:, :].tensor, offset=0, ap=[[0, P], [1, 16]]),
            )

            # PE "pre-observe" dummies: walrus fp32 fused matmul (LDW+MM)
            # tolerates only one sync wait, so make the PE observe the DMA
            # semaphores up front via tiny throwaway transposes.
            scr = ps_x.tile([2, 2], F32)
            nc.tensor.matmul(out=scr[:, :], lhsT=idn[:2, :2], rhs=idn[:2, :2],
                             is_transpose=True, start=True, stop=True,
                             skip_group_check=True)
            nc.tensor.matmul(out=scr[:, :], lhsT=wsb[:2, 0, :2], rhs=idn[:2, :2],
                             is_transpose=True, start=True, stop=True,
                             skip_group_check=True)

            xa = xin[:, :, :]

            # ---- stage A: powers of x, 1-x ----
            FD6 = S * DIM
            omx = singles.tile([P, S, DIM], F32)
            x2 = singles.tile([P, S, DIM], F32)
            x3 = singles.tile([P, S, DIM], F32)
            omx2 = singles.tile([P, S, DIM], F32)
            omx3 = singles.tile([P, S, DIM], F32)
            # omx = (x * -1) + 1
            nc.vector.tensor_scalar(
                out=omx[:, :, :], in0=xa, scalar1=-1.0, scalar2=1.0, op0=MUL, op1=ADD
            )
            nc.vector.tensor_tensor(out=x2[:, :, :], in0=xa, in1=xa, op=MUL)
            nc.vector.tensor_tensor(
                out=omx2[:, :, :], in0=omx[:, :, :], in1=omx[:, :, :], op=MUL
            )
            nc.vector.tensor_tensor(out=x3[:, :, :], in0=x2[:, :, :], in1=xa, op=MUL)
            nc.vector.tensor_tensor(
                out=omx3[:, :, :], in0=omx2[:, :, :], in1=omx[:, :, :], op=MUL
            )

            # ---- stage B: deg-3 tables Bbig[p, s, j, a]  j=0..4 ----
            NJ = 5
            Bbig = singles.tile([P, S, NJ, 4], F32)
            for (a, src, scl, other) in (
                (0, omx3, None, None),
                (1, xin, 3.0, omx2),
                (2, x2, 3.0, omx),
                (3, x3, None, None),
            ):
                src_ap = _ap(src[:, :, :], 0, [[DIM, S], [1, NJ]])
                out_ap = _ap(Bbig[:, :, :, :], a, [[4 * NJ, S], [4, NJ]])
                if scl is None:
                    nc.vector.tensor_copy(out=out_ap, in_=src_ap)
                else:
                    nc.vector.scalar_tensor_tensor(
                        out=out_ap, in0=src_ap, scalar=scl,
                        in1=_ap(other[:, :, :], 0, [[DIM, S], [1, NJ]]),
                        op0=MUL, op1=MUL,
                    )

            # ---- stage C+D: cond tiles, transpose, matmul per group ----
            tf_big = bigs.tile([P, S, M90], F32)   # natural-layout tf
            psi_big = bigs.tile([P, S, DIM, 15], F32)
            p15 = bigs.tile([P, S, DIM], F32)
            ebig = bigs.tile([P, S, DIM, 15], F32)
            sred = bigs.tile([P, S, DIM], F32)

            def emit_phipsi(j):
                px = powp.tile([P, 16, S], F32, tag="px")
                pq = powp.tile([P, 16, S], F32, tag="pq")
                for (tbl, base) in ((px, xin), (pq, omx)):
                    nc.vector.memset(tbl[:, 0, :], 1.0)
                    nc.vector.tensor_copy(
                        out=tbl[:, 1, :], in_=_ap(base[:, :, :], j, [[DIM, S]]))
                    t1 = tbl[:, :, :]
                    nc.vector.tensor_tensor(
                        out=tbl[:, 2, :], in0=t1[:, 1, :], in1=t1[:, 1, :], op=MUL)
                    nc.vector.tensor_tensor(
                        out=_ap(t1, 3 * S, [[1, 2 * S]]),
                        in0=_ap(t1, S, [[1, 2 * S]]),
                        in1=_ap(t1, 2 * S, [[0, 2], [1, S]]), op=MUL)
                    nc.any.tensor_tensor(
                        out=_ap(t1, 5 * S, [[1, 4 * S]]),
                        in0=_ap(t1, S, [[1, 4 * S]]),
                        in1=_ap(t1, 4 * S, [[0, 4], [1, S]]), op=MUL)
                    nc.any.tensor_tensor(
                        out=_ap(t1, 9 * S, [[1, 7 * S]]),
                        in0=_ap(t1, S, [[1, 7 * S]]),
                        in1=_ap(t1, 8 * S, [[0, 7], [1, S]]), op=MUL)
                v = powp.tile([P, 16, S], F32, tag="v")
                nc.any.tensor_tensor(
                    out=v[:, :, :],
                    in0=px[:, :, :],
                    in1=_ap(pq[:, :, :], 15 * S, [[-S, 16], [1, S]]), op=MUL)
                nc.any.tensor_tensor(
                    out=v[:, :, :], in0=v[:, :, :],
                    in1=_ap(kapt[:, :], 0, [[1, 16], [0, S]]), op=MUL)
                nc.any.tensor_tensor(
                    out=_ap(psi_big[:, :, :, :], j * 15, [[DIM * 15, S], [1, 15]]),
                    in0=_ap(v[:, :, :], 0, [[1, S], [S, 15]]),
                    in1=_ap(v[:, :, :], S, [[1, S], [S, 15]]), op=SUB)
                nc.vector.tensor_copy(
                    out=_ap(p15[:, :, :], j, [[DIM, S]]),
                    in_=_ap(v[:, :, :], 15 * S, [[1, S]]))

            def emit_combine(s0, s1):
                ns = s1 - s0
                nc.any.tensor_tensor(
                    out=_ap(ebig[:, :, :, :], s0 * M90, [[1, ns * M90]]),
                    in0=_ap(tf_big[:, :, :], s0 * M90, [[1, ns * M90]]),
                    in1=_ap(psi_big[:, :, :, :], s0 * M90, [[1, ns * M90]]), op=MUL)
                nc.vector.tensor_reduce(
                    out=_ap(sred[:, :, :], s0 * DIM, [[1, ns * DIM]]),
                    in_=_ap(ebig[:, :, :, :], s0 * M90, [[15, ns * DIM], [1, 15]]),
                    op=ADD, axis=mybir.AxisListType.X)

            for g in range(NG):
                ctb = ctbp.tile([P, KCH, NT, P], F32, tag="ctb")
                bb = Bbig[:, :, :, :]
                gb = g * NT * NJ * 4   # B-table offset of this group's tiles
                TS = NJ * 4            # per-tile stride in Bbig cols
                k2g = condp.tile([P, NT, 16], F32, tag="k2")
                k3g = condp.tile([P, NT, 64], F32, tag="k3")
                q34g = condp.tile([P, NT, 16], F32, tag="q34")
                nc.vector.tensor_tensor(
                    out=k2g[:, :, :],
                    in0=_ap(bb, gb + 0, [[TS, NT], [1, 4], [0, 4]]),
                    in1=_ap(bb, gb + 4, [[TS, NT], [0, 4], [1, 4]]), op=MUL)
                nc.vector.tensor_tensor(
                    out=k3g[:, :, :],
                    in0=_ap(k2g[:, :, :], 0, [[16, NT], [1, 16], [0, 4]]),
                    in1=_ap(bb, gb + 8, [[TS, NT], [0, 16], [1, 4]]), op=MUL)
                nc.vector.tensor_tensor(
                    out=q34g[:, :, :],
                    in0=_ap(bb, gb + 12, [[TS, NT], [1, 4], [0, 4]]),
                    in1=_ap(bb, gb + 16, [[TS, NT], [0, 4], [1, 4]]), op=MUL)
                for t in range(NT):
                    cnd = condp.tile([P, CDIM], F32, tag="cond")
                    nc.any.tensor_tensor(
                        out=cnd[:, :],
                        in0=_ap(k3g[:, :, :], t * 64, [[1, 64], [0, 16]]),
                        in1=_ap(q34g[:, :, :], t * 16, [[0, 64], [1, 16]]), op=MUL)

                    # transpose 8 128x128 blocks -> 2-bank psum tile
                    pst = ps_t.tile([P, KCH, P], F32, tag="pst")
                    # dummy absorbs the psum-slot-release wait so the first
                    # real transpose carries only the DVE (cond) wait
                    nc.tensor.matmul(out=pst[:2, 0, :2], lhsT=idn[:2, :2],
                                     rhs=idn[:2, :2], is_transpose=True,
                                     start=True, stop=True,
                                     skip_group_check=True)
                    for k in range(KCH):
                        nc.tensor.matmul(
                            out=pst[:, k, :],
                            lhsT=cnd[:, k * P:(k + 1) * P],
                            rhs=idn[:, :],
                            is_transpose=True,
                            start=(k % 4 == 0),
                            stop=(k % 4 == 3),
                        )
                    # copy psum -> condT sbuf (scatter over k, col t*128)
                    for half in range(2):
                        nc.scalar.copy(
                            out=_ap(ctb[:, :, :, :], (half * 4) * NT * P + t * P,
                                    [[NT * P, 4], [1, P]]),
                            in_=_ap(pst[:, :, :], half * 4 * P, [[P, 4], [1, P]]),
                        )

                # tf natural directly: stationary cond^T, moving W (90 cols)
                for t in range(NT):
                    pmm = ps_mm.tile([P, M90], F32, tag="pmm")
                    for k in range(KCH):
                        nc.tensor.matmul(
                            out=pmm[:, :],
                            lhsT=ctb[:, k, t, :],
                            rhs=wsb[:, k, :],
                            start=(k == 0),
                            stop=(k == KCH - 1),
                        )
                    nc.scalar.copy(
                        out=_ap(tf_big[:, :, :], (g * NT + t) * M90, [[1, M90]]),
                        in_=pmm[:, :],
                    )
                pp = {0: 0, 1: 1, 3: 2, 5: 3, 7: 4, 8: 5}
                if g in pp:
                    emit_phipsi(pp[g])
                elif g in (9, 11, 13, 15):
                    # psi complete after group 8; combine finished s-ranges
                    done = {9: (0, 24), 11: (24, 40), 13: (40, 56),
                            15: (56, 64)}[g]
                    emit_combine(*done)

            # ---- stage F: final combine tail ----
            nc.vector.tensor_tensor(
                out=sred[:, :, :], in0=sred[:, :, :], in1=p15[:, :, :], op=ADD)
            t1 = bigs.tile([P, S, 3], F32)
            nc.vector.tensor_tensor(
                out=t1[:, :, :],
                in0=_ap(sred[:, :, :], 0, [[DIM, S], [2, 3]]),
                in1=_ap(sred[:, :, :], 1, [[DIM, S], [2, 3]]), op=MUL)
            dq = bigs.tile([P, S], F32)
            nc.vector.tensor_tensor(
                out=dq[:, :],
                in0=_ap(t1[:, :, :], 0, [[3, S]]),
                in1=_ap(t1[:, :, :], 1, [[3, S]]), op=MUL)
            nc.vector.tensor_tensor(
                out=dq[:, :], in0=dq[:, :],
                in1=_ap(t1[:, :, :], 2, [[3, S]]), op=MUL)
            nc.sync.dma_start(out=dens_out[:, :], in_=dq[:, :])

    nc.finalize()
    return nc


def _softplus64(v):
    return np.logaddexp(0.0, v)


def _host_w(As):
    cols = []
    for i in range(DIM):
        c = np.cumsum(_softplus64(As[i].astype(np.float64)), axis=1)
        ca = 2.0 * (1.0 / (1.0 + np.exp(-c)) - 0.5)
        cols.append(np.repeat(ca, 4 ** (5 - i), axis=0))
    return np.concatenate(cols, axis=1).astype(np.float32)


def kernel(**inputs):
    x = np.asarray(inputs["x"], dtype=np.float32)
    As = [np.asarray(inputs[f"A{i}"], dtype=np.float32) for i in range(DIM)]

    if "nc" not in _CACHE:
        _CACHE["nc"] = _build_nc()
    nc = _CACHE["nc"]

    w = _host_w(As)
    kapv = (16.0 * np.array([math.comb(15, a) for a in range(16)],
                            dtype=np.float64)).astype(np.float32)[None, :]
    idn = np.eye(P, dtype=np.float32)

    in_maps = []
    for c in range(NCORES):
        xc = x[c * NC:(c + 1) * NC].reshape(P, S, DIM)
        in_maps.append({"xr": xc, "wmat": w, "kap": kapv, "ident": idn})

    res = run_bass_kernel_spmd(nc, in_maps, core_ids=list(range(NCORES)))
    outs = [r["dens"].reshape(NC) for r in res.results]
    return np.concatenate(outs, axis=0)


if __name__ == "__main__":
    rng = np.random.default_rng(0)
    ins = {"x": rng.uniform(0, 1, (N, DIM)).astype(np.float32)}
    for i in range(DIM):
        ins[f"A{i}"] = rng.uniform(0, 1, ((4 ** i), 15)).astype(np.float32)
    out = kernel(**ins)
    print(out.shape, out[:4])
